# revision 61
# baseline (speedup 1.0000x reference)
"""TRN2 Bass kernel for nn_AttentionBlock (GroupNorm32 + 8-head attention + proj + residual).

Sharding: data-parallel over batch — batch=8, one batch element per NeuronCore, no
collectives.

v4 structure (the 64 score-exp activations on ACT are the ~73us floor; everything
else is arranged to keep that stream dense):
- All input DMAs dispatch from the sync engine (each dma_start costs ~0.7us of
  the issuing engine's time — keep them off ACT/DVE), priority-interleaved:
  x chunk / q+k weight chunk pairs first, then v columns, then proj weights.
  x is held in bf16 (halves critical input bytes; residual quantization ~2e-3
  rel, well inside the 2e-2 gate).
- GroupNorm stats per chunk: sum-of-squares on ACT (Square+accum_out), sum on
  DVE, group fold via tiny gmask matmuls, rsqrt as exp(-0.5*ln v), xhat on DVE
  tensor_scalar. Pair-0 q/k matmuls run k-outer interleaved with the stats
  chunks; junk matmuls keep the PE HAM clock warm through the gaps (the PE
  re-throttles to 1.2 GHz after ~3.4us idle).
- Attention as a flat (pair, sc) stream: scores+exps first in each iteration;
  the previous pair's tail work (attn sc6-7, staging, the whole t=1 attn pass,
  divisions) is hoisted BEHIND the next pair's first scores+exps so the exp
  stream never waits on it. attn matmuls run lag-2 behind scores in two
  t-passes (t=0 in-loop, t=1 as a pair-end block). vT and later pairs' q/k
  matmuls are emitted as in-loop PE fillers (2 spare PSUM banks). ew tiles are
  double-pair buffered (bufs=32) so exps never wait on t=1-pass readers.
- Softmax denominators ride a ones-column in vT through the attn matmul;
  reciprocals on DVE (InstReciprocal), division = gpsimd partition_broadcast +
  DVE multiply, pipelined into the following pair.
- Tail: pair-3's t=0 division overlaps its t=1 attn pass; proj runs k-inner
  per output tile so tiles stream out add+DMA one by one (both DMA queues) —
  output bandwidth and the division chain finish together.

Numerics: all matmuls bf16 with fp32 PSUM accumulation; everything else fp32.

Self-contained: hardcodes shapes from the problem spec (x (8,512,32,32) f32 etc).
"""
import numpy as np
import ml_dtypes

B, CH, HH, WW = 8, 512, 32, 32
L = HH * WW                  # 1024
HEADS = 8
GROUPS = 32
EPS = 1e-5
DH = CH // HEADS             # 64
KC = CH // 128               # 4 c-chunks
SC = L // 128                # 8 s/l-chunks
TC = L // 512                # 2 t-chunks
GN_N = (CH // GROUPS) * L    # elements per group = 16384

_cache = {}


def _build(has_qkv_bias, has_proj_bias):
    import concourse.bass as bass
    import concourse.tile as tile
    from concourse import bacc, mybir
    import bass_rust as _bass_rust
    from concourse.hw_specs import get_activation_tables

    F32 = mybir.dt.float32
    BF16 = mybir.dt.bfloat16
    F8 = mybir.dt.float8e4
    DR = mybir.MatmulPerfMode.DoubleRow
    AF = mybir.ActivationFunctionType
    OP = mybir.AluOpType
    AX = mybir.AxisListType

    class _Bacc(bacc.Bacc):
        # Pin Exp/Ln to the combined `natural_log_exp_and_others` table set so
        # alternating Ln/Exp activations don't thrash ACT_TABLE_LOADs (~2.7us
        # each).
        def insert_act_table_loads(self):
            has_activation = any(
                isinstance(i, mybir.InstActivation)
                for b in self.main_func.blocks
                for i in b.instructions
            )
            if not has_activation:
                return
            combo = {AF.Exp, AF.Ln}
            tables = []
            for name, fns in get_activation_tables(self.m.arch).items():
                if name != "natural_log_exp_and_others":
                    fns = {f for f in fns if f not in combo}
                tables.append((name, fns))
            _bass_rust.insert_act_table_loads(self, tables)

    nc = _Bacc("TRN2", target_bir_lowering=False, debug=False, num_devices=8)

    x_d = nc.dram_tensor("x", [CH, L], BF16, kind="ExternalInput").ap()
    qw_d = nc.dram_tensor("qkv_wt", [CH, 3 * CH], BF16, kind="ExternalInput").ap()
    pw_d = nc.dram_tensor("proj_wt", [CH, CH], F8, kind="ExternalInput").ap()
    gmask_d = nc.dram_tensor("gmask", [128, 8], F32, kind="ExternalInput").ap()
    gmaskT_d = nc.dram_tensor("gmask_t", [8, 128], F32, kind="ExternalInput").ap()
    if has_qkv_bias:
        qkb_d = nc.dram_tensor("qk_bias", [128, 8], F32, kind="ExternalInput").ap()
        vb_d = nc.dram_tensor("v_bias", [128, KC], F32, kind="ExternalInput").ap()
    if has_proj_bias:
        pb_d = nc.dram_tensor("p_bias", [128, KC], F32, kind="ExternalInput").ap()
    out_d = nc.dram_tensor("out", [CH, L], F32, kind="ExternalOutput").ap()

    with tile.TileContext(nc) as tc:
        import contextlib
        ctx = contextlib.ExitStack()
        pers = ctx.enter_context(tc.tile_pool(name="pers", bufs=1))
        scr = ctx.enter_context(tc.tile_pool(name="scr", bufs=2))
        ewp = ctx.enter_context(tc.tile_pool(name="ewp", bufs=16))
        dvp = ctx.enter_context(tc.tile_pool(name="dvp", bufs=2))
        asg = ctx.enter_context(tc.tile_pool(name="asg", bufs=8))
        outp = ctx.enter_context(tc.tile_pool(name="outp", bufs=3))

        # ---- input DMAs fanned over 4 engine queues (x chunks first so the
        # GN stats pipeline starts asap; qw q/k next; v and pw trail on
        # sync). Each engine's dma_start costs ~0.6us of that engine's time,
        # but ACT/DVE/Pool are idle until the first x chunk lands anyway. ----
        xs = pers.tile([128, KC * L], BF16, tag="xs")
        qw = pers.tile([128, KC * 3 * CH], BF16, tag="qw")
        pw = pers.tile([128, KC * CH], F8, tag="pw")
        gmask = pers.tile([128, 8], F32, tag="gmask")
        gmaskT = pers.tile([8, 128], F32, tag="gmask_t")
        if has_qkv_bias:
            qkb = pers.tile([128, 8], F32, tag="qkb")
            vb = pers.tile([128, KC], F32, tag="vb")
        if has_proj_bias:
            pb = pers.tile([128, KC], F32, tag="pb")
        qeng = [nc.sync, nc.scalar, nc.gpsimd, nc.sync]
        for k in range(KC):
            qeng[k].dma_start(xs[:, k * L:(k + 1) * L],
                              x_d[128 * k:128 * (k + 1), :])
        for k in range(KC):
            qeng[k].dma_start(qw[:, k * 3 * CH:k * 3 * CH + 1024],
                              qw_d[128 * k:128 * (k + 1), 0:1024])
        nc.sync.dma_start(gmask[:], gmask_d[:])
        nc.sync.dma_start(gmaskT[:], gmaskT_d[:])
        if has_qkv_bias:
            nc.sync.dma_start(qkb[:], qkb_d[:])
            nc.sync.dma_start(vb[:], vb_d[:])
        if has_proj_bias:
            nc.sync.dma_start(pb[:], pb_d[:])
        for k in range(KC):   # v columns (vT runs inside pair 0)
            nc.sync.dma_start(qw[:, k * 3 * CH + 1024:(k + 1) * 3 * CH],
                              qw_d[128 * k:128 * (k + 1), 1024:1536])
        for k in range(KC):
            nc.sync.dma_start(pw[:, k * CH:(k + 1) * CH],
                              pw_d[128 * k:128 * (k + 1), :])

        # ---- small constants + ACT table-load trigger ----
        epsb = pers.tile([8, 1], F32, tag="epsb")
        nc.gpsimd.memset(epsb[:], EPS)
        tldt = pers.tile([8, 1], F32, tag="tldt")
        nc.scalar.activation(tldt[:], epsb[:], AF.Exp)

        wsrc = pers.tile([128, 640], BF16, tag="wsrc")
        nc.gpsimd.memset(wsrc[:], 0.0)
        head_psum = tc.tile_pool(name="psH", bufs=1, space="PSUM")
        psH = head_psum.__enter__()
        jps = psH.tile([128, 512], F32, tag="junk", bufs=1)

        def junk(n):
            for _ in range(n):
                nc.tensor.matmul(jps[:], wsrc[:, 0:128], wsrc[:, 128:640],
                                 start=True, stop=True)

        junk(14)

        # ---- GroupNorm stats + xhat per chunk, pair-0 q/k k-outer inline ----
        stat = pers.tile([128, 8], F32, tag="stat")
        xhat = pers.tile([128, KC * L], BF16, tag="xhat")
        bc = pers.tile([128, 2 * KC], F32, tag="bc")
        qk = pers.tile([128, 8 * L], BF16, tag="qk")

        psk = {}
        for j in (0, 4):
            for t in range(TC):
                psk[(j, t)] = psH.tile([128, 512], F32, tag="qkps", bufs=4,
                                       name=f"psk{j}_{t}")

        for k in range(KC):
            xk = xs[:, k * L:(k + 1) * L]
            sq = scr.tile([128, L], F32, tag="sq")
            nc.scalar.activation(sq[:], xk, AF.Square,
                                 accum_out=stat[:, 2 * k + 1:2 * k + 2])
            nc.vector.tensor_reduce(stat[:, 2 * k:2 * k + 1], xk,
                                    axis=AX.X, op=OP.add)
            gst_ps = psH.tile([8, 2], F32, tag="gst", bufs=2)
            nc.tensor.matmul(gst_ps[:], gmask[:], stat[:, 2 * k:2 * k + 2],
                             start=True, stop=True)
            s2k = pers.tile([8, 2], F32, tag=f"s2k{k}")
            vk = pers.tile([8, 2], F32, tag=f"vk{k}")
            nc.vector.tensor_scalar_mul(s2k[:], gst_ps[:], 1.0 / GN_N)
            nc.vector.tensor_mul(vk[:, 1:2], s2k[:, 0:1], s2k[:, 0:1])
            nc.vector.tensor_sub(vk[:, 0:1], s2k[:, 1:2], vk[:, 1:2])
            nc.scalar.activation(vk[:, 1:2], vk[:, 0:1], AF.Ln, bias=epsb[:])
            nc.scalar.activation(s2k[:, 1:2], vk[:, 1:2], AF.Exp, scale=-0.5)
            bc_ps = psH.tile([128, 2], F32, tag="bcps", bufs=1)
            nc.tensor.matmul(bc_ps[:], gmaskT[:], s2k[:], start=True, stop=True)
            nc.vector.tensor_copy(bc[:, 2 * k:2 * k + 2], bc_ps[:])
            nmr = pers.tile([128, 1], F32, tag=f"nmr{k}")
            nc.vector.tensor_scalar(
                out=nmr[:], in0=bc[:, 2 * k:2 * k + 1],
                scalar1=bc[:, 2 * k + 1:2 * k + 2], scalar2=-1.0,
                op0=OP.mult, op1=OP.mult)
            nc.vector.tensor_scalar(
                out=xhat[:, k * L:(k + 1) * L], in0=xk,
                scalar1=bc[:, 2 * k + 1:2 * k + 2], scalar2=nmr[:],
                op0=OP.mult, op1=OP.add)
            for j in (0, 4):
                for t in range(TC):
                    nc.tensor.matmul(
                        psk[(j, t)][:],
                        qw[:, k * 3 * CH + 128 * j:k * 3 * CH + 128 * (j + 1)],
                        xhat[:, k * L + 512 * t:k * L + 512 * (t + 1)],
                        start=(k == 0), stop=(k == KC - 1))
            junk(2)

        def qk_store(j, t, ps_ap, eng):
            dst = qk[:, j * L + 512 * t:j * L + 512 * (t + 1)]
            if eng == "s":
                if has_qkv_bias:
                    nc.scalar.activation(dst, ps_ap, AF.Identity,
                                         bias=qkb[:, j:j + 1])
                else:
                    nc.scalar.activation(dst, ps_ap, AF.Identity)
            else:
                if has_qkv_bias:
                    nc.vector.tensor_scalar_add(dst, ps_ap, qkb[:, j:j + 1])
                else:
                    nc.vector.tensor_copy(dst, ps_ap)

        qk_store(0, 0, psk[(0, 0)][:], "v")
        qk_store(4, 0, psk[(4, 0)][:], "s")
        qk_store(0, 1, psk[(0, 1)][:], "v")
        qk_store(4, 1, psk[(4, 1)][:], "s")
        head_psum.__exit__(None, None, None)

        # ---- attention phase pools: scores 4 banks, attn accumulators 2
        # banks (two t-passes), filler accumulators 2 banks.
        # v/ew are fp8e4: exp(score-4) fits e4m3 (|score|<6.3), the -4 shift
        # cancels in the softmax division, and fp8 enables DoubleRow attn
        # matmuls (2 sc-chunks contracted per matmul at 0.5 cyc/row). ----
        a_sb = pers.tile([128, KC * L], F8, tag="a_sb")
        # vt layout: [scp][head][sc-even 80 | sc-odd 80] (65 used + 15 pad
        # per group) — dual-fp8 LDWEIGHTS needs the two contraction groups at
        # an even, 16B-aligned stride
        vt = pers.tile([128, (SC // 2) * HEADS * 160], F8, tag="vt")
        for scp in range(SC // 2):
            v4 = vt[:, scp * 1280:(scp + 1) * 1280].rearrange(
                "p (h s c) -> p h s c", h=8, s=2, c=80)
            nc.gpsimd.memset(v4[:, :, :, 64:65], 1.0)
        nb4 = pers.tile([128, 1], F32, tag="nb4")
        nc.gpsimd.memset(nb4[:], -4.0)

        attn_psum = tc.tile_pool(name="psS", bufs=2, space="PSUM")
        psS = attn_psum.__enter__()
        attn_acc = tc.tile_pool(name="psA", bufs=2, space="PSUM")
        psA = attn_acc.__enter__()
        fill_psum = tc.tile_pool(name="psF", bufs=2, space="PSUM")
        psF = fill_psum.__enter__()

        vt_ps = {}

        def emit_vt_q(lc, k):
            # vT for one l-chunk, one k-chunk matmul per quarter emission
            if k == 0:
                vt_ps[lc] = psF.tile([128, 512], F32, tag="f", bufs=2,
                                     name=f"vtps{lc}")
            ps = vt_ps[lc]
            nc.tensor.matmul(
                ps[:], xhat[:, k * L + 128 * lc:k * L + 128 * (lc + 1)],
                qw[:, k * 3 * CH + 2 * CH:k * 3 * CH + 3 * CH],
                start=(k == 0), stop=(k == KC - 1))
            if k == KC - 1:
                # cast to fp8 on the way into vt ([scp][h][so 80|80] layout)
                scp, so = divmod(lc, 2)
                v4_ = vt[:, scp * 1280:(scp + 1) * 1280].rearrange(
                    "p (hh s c) -> p hh s c", hh=8, s=2, c=80)
                src = ps[:].rearrange("p (hh c) -> p hh c", c=64)
                nc.vector.tensor_copy(v4_[:, :, so, 0:64], src)

        fq_ps = {}

        def emit_qk_q(j, k):
            # one k-chunk (both t halves) of a later-pair q/k o-chunk per
            # quarter emission; stores ride the last quarter
            if k == 0:
                fq_ps[j] = [psF.tile([128, 512], F32, tag="f", bufs=2,
                                     name=f"fq{j}_{t}") for t in range(TC)]
            for t in range(TC):
                nc.tensor.matmul(
                    fq_ps[j][t][:],
                    qw[:, k * 3 * CH + 128 * j:k * 3 * CH + 128 * (j + 1)],
                    xhat[:, k * L + 512 * t:k * L + 512 * (t + 1)],
                    start=(k == 0), stop=(k == KC - 1))
            if k == KC - 1:
                for t in range(TC):
                    qk_store(j, t, fq_ps[j][t][:], "v")

        def V(lc, k):
            return lambda: emit_vt_q(lc, k)

        def Q(j, k):
            return lambda: emit_qk_q(j, k)

        fillers = {
            (0, 1): [V(0, 0), V(0, 1), V(0, 2), V(0, 3), V(1, 0), V(1, 1)],
            (0, 2): [V(1, 2), V(1, 3), V(2, 0), V(2, 1), V(2, 2), V(2, 3)],
            (0, 3): [V(3, 0), V(3, 1), V(3, 2), V(3, 3)],
            (0, 4): [V(4, 0), V(4, 1), V(4, 2), V(4, 3), Q(1, 0)],
            (0, 5): [V(5, 0), V(5, 1), V(5, 2), V(5, 3), Q(1, 1), Q(1, 2)],
            (0, 6): [V(6, 0), V(6, 1), V(6, 2), V(6, 3), Q(1, 3), Q(5, 0)],
            (0, 7): [V(7, 0), V(7, 1), V(7, 2), V(7, 3), Q(5, 1), Q(5, 2),
                     Q(5, 3)],
            (1, 3): [Q(2, 0)],
            (1, 4): [Q(2, 1)],
            (1, 5): [Q(2, 2), Q(6, 0)],
            (1, 6): [Q(2, 3), Q(6, 1)],
            (1, 7): [Q(6, 2), Q(6, 3)],
            (2, 3): [Q(3, 0)],
            (2, 4): [Q(3, 1)],
            (2, 5): [Q(3, 2), Q(7, 0)],
            (2, 6): [Q(3, 3), Q(7, 1)],
            (2, 7): [Q(7, 2), Q(7, 3)],
        }

        def q_ap(m, e, t):
            return qk[64 * e:64 * (e + 1), m * L + 512 * t:m * L + 512 * (t + 1)]

        def k_ap(m, e, sc):
            return qk[64 * e:64 * (e + 1),
                      (4 + m) * L + 128 * sc:(4 + m) * L + 128 * (sc + 1)]

        # ew4[(m, scp, e)]: [128, 2048] fp8 tile for the sc pair {2scp, 2scp+1}
        # laid out [scE-t0 | scO-t0 | scE-t1 | scO-t1] so a DoubleRow attn
        # matmul reads a contiguous [128, 2, 512] rhs per t
        ew_tiles = {}

        def emit_scores(m, sc):
            scp, so = divmod(sc, 2)
            ps_w = [None, None]
            for e in range(2):
                pw_t = psS.tile([128, 1024], F32, tag="ps")
                ps_w[e] = pw_t
            for e in range(2):
                for t in range(TC):
                    nc.tensor.matmul(ps_w[e][:, 512 * t:512 * (t + 1)],
                                     k_ap(m, e, sc), q_ap(m, e, t),
                                     start=True, stop=True)
            for e in range(2):
                if so == 0:
                    ew_tiles[(m, scp, e)] = ewp.tile(
                        [128, 2 * L], F8, tag="ew", name=f"ew{m}_{scp}_{e}")
                ew4 = ew_tiles[(m, scp, e)]
                dst = ew4.rearrange("p (t s c) -> p t s c", t=2, s=2,
                                    c=512)[:, :, so, :]
                nc.scalar.activation(dst, ps_w[e][:], AF.Exp, bias=nb4[:])

        pa = {}
        psA3 = [None]  # pair-3 t1 accumulator pool, opened once psF retires

        def attn_dr(m, tp, scp, e):
            # DoubleRow fp8 matmul: contracts sc-chunks 2scp and 2scp+1 at
            # once (vt lhsT [128, 2, 65], ew rhs [128, 2, 512])
            if (m, tp) not in pa:
                if (m, tp) == (3, 1):
                    pa[(m, tp)] = [psA3[0].tile([65, 512], F32, tag="pa3",
                                                name=f"pa3_1_{ee}")
                                   for ee in range(2)]
                else:
                    pa[(m, tp)] = [psA.tile([65, 512], F32, tag="pa",
                                            name=f"pa{m}_{tp}_{ee}")
                                   for ee in range(2)]
            h0 = scp * 1280 + (2 * m + e) * 160
            lhsT = vt[:, h0:h0 + 160].rearrange(
                "p (s c) -> p s c", c=80)[:, :, 0:65]
            rhs = ew_tiles[(m, scp, e)][:, tp * 1024:(tp + 1) * 1024].rearrange(
                "p (s c) -> p s c", s=2)
            nc.tensor.matmul(
                pa[(m, tp)][e][:], lhsT, rhs,
                start=(scp == 0), stop=(scp == SC // 2 - 1),
                perf_mode=DR)

        def div_recip2(stgs2, engs=None):
            # copy ones-row to partition 0, then approx reciprocal (both ops
            # require partition-0-aligned operands); no DMA round trip
            rdens = []
            for i, (sg, e, t, mm_) in enumerate(stgs2):
                den = dvp.tile([1, 512], F32, tag="den", bufs=4,
                               name=f"den{mm_}_{t}_{e}")
                nc.vector.tensor_copy(den[:], sg[64:65, :])
                r = dvp.tile([1, 512], F32, tag="rden", bufs=4,
                             name=f"r{mm_}_{t}_{e}")
                nc.vector.reciprocal_approx_fast(r[:], den[:])
                rdens.append(r)
            return rdens

        def div_mul(rdens, i, sg, e, t, mm_, mul_eng="v"):
            bsb = dvp.tile([64, 512], F32, tag="bsb", bufs=4)
            nc.gpsimd.partition_broadcast(bsb[:], rdens[i][0:1, :])
            dst = a_sb[64 * e:64 * (e + 1),
                       mm_ * L + 512 * t:mm_ * L + 512 * (t + 1)]
            eng = nc.vector if mul_eng == "v" else nc.gpsimd
            eng.tensor_mul(dst, sg[0:64, :], bsb[:])
            if has_qkv_bias:
                nc.vector.tensor_scalar_add(
                    dst, dst, vb[64 * e:64 * (e + 1), mm_:mm_ + 1])

        def stage(m, tp, engs=("v", "v")):
            out = []
            for e in range(2):
                sg = asg.tile([65, 512], F32, tag="astg", name=f"sg{m}_{tp}_{e}")
                if engs[e] == "s":
                    nc.scalar.activation(sg[:], pa[(m, tp)][e][:], AF.Identity)
                else:
                    nc.vector.tensor_copy(sg[:], pa[(m, tp)][e][:])
                out.append((sg, e, tp, m))
            return out

        def tail_steps(m):
            # previous pair's tail, spread one step per (m+1, sc) slot so it
            # never lumps in the PE FIFO ahead of the next pair's scores
            for e in range(2):
                attn_dr(m, 0, 3, e)
            stgs_t0 = stage(m, 0)
            yield
            for scp in (0, 1):
                for e in range(2):
                    attn_dr(m, 1, scp, e)
            yield
            for scp in (2, 3):
                for e in range(2):
                    attn_dr(m, 1, scp, e)
            stgs_t1 = stage(m, 1)
            yield
            rden0 = div_recip2(stgs_t0)
            yield
            for i, s in enumerate(stgs_t0):
                div_mul(rden0, i, *s)
            yield
            rden1 = div_recip2(stgs_t1)
            yield
            for i, s in enumerate(stgs_t1):
                div_mul(rden1, i, *s)
            yield

        attn_acc3 = None
        pending_tail = None
        for g in range(32):
            m, sc = divmod(g, 8)
            if g == 24:
                # fillers are done (last at (2,6)); retire psF and reuse its
                # 2 banks for pair-3's t1 accumulators so the t1 attn pass
                # runs lag-2 in-loop instead of entirely after the last exp
                fill_psum.__exit__(None, None, None)
                attn_acc3 = tc.tile_pool(name="psA3", bufs=2, space="PSUM")
                psA3[0] = attn_acc3.__enter__()
            emit_scores(m, sc)
            if sc == 0:
                if pending_tail is not None:
                    for _ in pending_tail:
                        pass
                pending_tail = tail_steps(m - 1) if m >= 1 else None
            if pending_tail is not None:
                next(pending_tail, None)
            for f in fillers.get((m, sc), ()):
                f()
            if sc in (3, 5, 7):
                scp = (sc - 3) // 2
                for e in range(2):
                    attn_dr(m, 0, scp, e)
                if m == 3:
                    for e in range(2):
                        attn_dr(m, 1, scp, e)

        # ---- pair-3 tail: both t-passes already ran lag in-loop; only
        # sc-pair 3 remains. No staging — numerators and the ones-row are
        # read straight from the attn PSUM accumulators; the den/recip/
        # bcast/mul chain is software-pipelined across the (tp, e) items. ----
        if pending_tail is not None:
            for _ in pending_tail:
                pass
        for e in range(2):
            attn_dr(3, 0, 3, e)
        for e in range(2):
            attn_dr(3, 1, 3, e)

        items = [(0, 0), (0, 1), (1, 0), (1, 1)]   # (tp, e), t0 first (proj t0)
        rs, bs = {}, {}

        def t3_den(tp, e):
            den = dvp.tile([1, 512], F32, tag="den", bufs=4,
                           name=f"d3_{tp}_{e}")
            nc.vector.tensor_copy(den[:], pa[(3, tp)][e][64:65, :])
            r = dvp.tile([1, 512], F32, tag="rden", bufs=4, name=f"r3_{tp}_{e}")
            nc.vector.reciprocal_approx_fast(r[:], den[:])
            rs[(tp, e)] = r

        def t3_bcast(tp, e):
            bsb = dvp.tile([64, 512], F32, tag="bsb", bufs=4,
                           name=f"b3_{tp}_{e}")
            nc.gpsimd.partition_broadcast(bsb[:], rs[(tp, e)][0:1, :])
            bs[(tp, e)] = bsb

        def t3_mul(tp, e):
            dst = a_sb[64 * e:64 * (e + 1),
                       3 * L + 512 * tp:3 * L + 512 * (tp + 1)]
            nc.vector.tensor_mul(dst, pa[(3, tp)][e][0:64, :], bs[(tp, e)][:])
            if has_qkv_bias:
                nc.vector.tensor_scalar_add(
                    dst, dst, vb[64 * e:64 * (e + 1), 3:4])

        # all dens+recips first (bcasts overlap the later DVE ops), then muls
        t3_den(0, 0)
        t3_bcast(0, 0)
        t3_den(0, 1)
        t3_bcast(0, 1)
        t3_den(1, 0)
        t3_bcast(1, 0)
        t3_den(1, 1)
        t3_bcast(1, 1)
        t3_mul(0, 0)
        t3_mul(0, 1)
        t3_mul(1, 0)
        t3_mul(1, 1)
        if attn_acc3 is not None:
            attn_acc3.__exit__(None, None, None)
        attn_acc.__exit__(None, None, None)
        attn_psum.__exit__(None, None, None)

        # ---- proj + residual. The k=0..2 partial accumulations only need
        # pairs 0-2 of a_sb (divided in-stream long ago), so they run DURING
        # the pair-3 division window — this keeps the PE HAM clock warm and
        # leaves only the k=3 finisher per tile gated on pair-3's muls. All 8
        # accumulators live at once (PSUM is free after the attn pools). ----
        # bufs=4: exactly the 4 banks the score pool vacated at the last exp
        # — overlapping the attn-accumulator banks would make the first proj
        # write wait for pair-3's division reads (tile-level WAR)
        with tc.tile_pool(name="psP", bufs=4, space="PSUM") as psP:
            ps_tiles = {}

            def proj_k012(t):
                for i in range(KC):
                    ps = psP.tile([128, 512], F32, tag="ps", name=f"psp{t}_{i}")
                    ps_tiles[(t, i)] = ps
                    pw_k = pw.rearrange("p (kk c) -> p kk c", c=CH)
                    ab_k = a_sb.rearrange("p (kk c) -> p kk c", c=L)
                    lhsT = pw_k[:, 0:2, 128 * i:128 * (i + 1)]
                    rhs = ab_k[:, 0:2, 512 * t:512 * (t + 1)]
                    nc.tensor.matmul(ps[:], lhsT, rhs,
                                     start=True, stop=False, perf_mode=DR)

            idx = 0

            def proj_fin(t):
                nonlocal idx
                kp = 2
                pw_k = pw.rearrange("p (kk c) -> p kk c", c=CH)
                ab_k = a_sb.rearrange("p (kk c) -> p kk c", c=L)
                for i in range(KC):
                    ps = ps_tiles[(t, i)]
                    lhsT = pw_k[:, kp:kp + 2, 128 * i:128 * (i + 1)]
                    rhs = ab_k[:, kp:kp + 2, 512 * t:512 * (t + 1)]
                    nc.tensor.matmul(ps[:], lhsT, rhs,
                                     start=False, stop=True, perf_mode=DR)
                    ot = outp.tile([128, 512], F32, tag="ot")
                    nc.vector.tensor_add(ot[:],
                                         xs[:, i * L + 512 * t:i * L + 512 * (t + 1)],
                                         ps[:])
                    if has_proj_bias:
                        nc.vector.tensor_scalar_add(ot[:], ot[:], pb[:, i:i + 1])
                    eng = (nc.sync, nc.scalar)[idx % 2]
                    eng.dma_start(
                        out_d[128 * i:128 * (i + 1), 512 * t:512 * (t + 1)], ot[:])
                    idx += 1

            proj_k012(0)
            proj_k012(1)
            proj_fin(0)
            proj_fin(1)
        ctx.close()

    nc.compile()
    return nc


def _prep_inputs(x, norm_w, norm_b, qkv_w, qkv_b, proj_w, proj_b):
    scale = DH ** -0.25
    w_eff = (qkv_w.astype(np.float64) * norm_w.astype(np.float64)[None, :])
    b_eff = qkv_b.astype(np.float64) + w_eff @ norm_b.astype(np.float64)
    perm = np.concatenate([
        np.concatenate([np.arange(h * 3 * DH + t * DH, h * 3 * DH + (t + 1) * DH)
                        for h in range(HEADS)])
        for t in range(3)])
    w_eff = w_eff[perm]
    b_eff = b_eff[perm]
    w_eff[:2 * CH] *= scale
    b_eff[:2 * CH] *= scale
    qkv_wt = np.ascontiguousarray(w_eff.T).astype(np.float32).astype(
        ml_dtypes.bfloat16)
    proj_wt = np.ascontiguousarray(proj_w.T).astype(np.float32).astype(
        ml_dtypes.float8_e4m3)

    p = np.arange(128)
    gmask = (p[:, None] // 16 == np.arange(8)[None, :]).astype(np.float32)
    gmask_t = np.ascontiguousarray(gmask.T)

    has_qkv_bias = bool(np.any(b_eff != 0.0))
    has_proj_bias = bool(np.any(proj_b != 0.0))
    common = {"qkv_wt": qkv_wt, "proj_wt": proj_wt, "gmask": gmask,
              "gmask_t": gmask_t}
    if has_qkv_bias:
        qk_part = b_eff[:2 * CH].astype(np.float32).reshape(8, 128).T
        v_part = b_eff[2 * CH:].astype(np.float32).reshape(KC, 128).T
        common["qk_bias"] = np.ascontiguousarray(qk_part)
        common["v_bias"] = np.ascontiguousarray(v_part)
    if has_proj_bias:
        common["p_bias"] = np.ascontiguousarray(
            proj_b.astype(np.float32).reshape(KC, 128).T)
    xf = np.ascontiguousarray(x.reshape(B, CH, L)).astype(np.float32)
    xf16 = xf.astype(ml_dtypes.bfloat16)
    in_maps = [dict(common, x=np.ascontiguousarray(xf16[i])) for i in range(B)]
    return in_maps, has_qkv_bias, has_proj_bias


def _get_nc(flags):
    if flags not in _cache:
        _cache[flags] = _build(*flags)
    return _cache[flags]


def _run(inputs, trace=False, tmpdir=None):
    import time
    from concourse.bass_utils import run_bass_kernel_spmd
    in_maps, hqb, hpb = _prep_inputs(**inputs)
    nc = _get_nc((hqb, hpb))
    kw = {}
    if trace:
        kw = dict(trace=True, tmpdir=tmpdir)
    last_err = None
    for attempt in range(3):
        try:
            res = run_bass_kernel_spmd(nc, in_maps, list(range(B)), **kw)
            break
        except Exception as e:  # noqa: BLE001
            last_err = e
            time.sleep(5)
    else:
        raise last_err
    out = np.stack([res.results[i]["out"] for i in range(B)])
    return out.reshape(B, CH, HH, WW).astype(np.float32), res


def kernel(x, norm_w, norm_b, qkv_w, qkv_b, proj_w, proj_b):
    out, _ = _run(dict(x=x, norm_w=norm_w, norm_b=norm_b, qkv_w=qkv_w,
                       qkv_b=qkv_b, proj_w=proj_w, proj_b=proj_b))
    return out



# revision 62
# speedup vs baseline: 1.2250x; 1.2250x over previous
"""TRN2 Bass kernel for nn_AttentionBlock (GroupNorm32 + 8-head attention + proj + residual).

Sharding: data-parallel over batch — batch=8, one batch element per NeuronCore, no
collectives.

v4 structure (the 64 score-exp activations on ACT are the ~73us floor; everything
else is arranged to keep that stream dense):
- All input DMAs dispatch from the sync engine (each dma_start costs ~0.7us of
  the issuing engine's time — keep them off ACT/DVE), priority-interleaved:
  x chunk / q+k weight chunk pairs first, then v columns, then proj weights.
  x is held in bf16 (halves critical input bytes; residual quantization ~2e-3
  rel, well inside the 2e-2 gate).
- GroupNorm stats per chunk: sum-of-squares on ACT (Square+accum_out), sum on
  DVE, group fold via tiny gmask matmuls, rsqrt as exp(-0.5*ln v), xhat on DVE
  tensor_scalar. Pair-0 q/k matmuls run k-outer interleaved with the stats
  chunks; junk matmuls keep the PE HAM clock warm through the gaps (the PE
  re-throttles to 1.2 GHz after ~3.4us idle).
- Attention as a flat (pair, sc) stream: scores+exps first in each iteration;
  the previous pair's tail work (attn sc6-7, staging, the whole t=1 attn pass,
  divisions) is hoisted BEHIND the next pair's first scores+exps so the exp
  stream never waits on it. attn matmuls run lag-2 behind scores in two
  t-passes (t=0 in-loop, t=1 as a pair-end block). vT and later pairs' q/k
  matmuls are emitted as in-loop PE fillers (2 spare PSUM banks). ew tiles are
  double-pair buffered (bufs=32) so exps never wait on t=1-pass readers.
- Softmax denominators ride a ones-column in vT through the attn matmul;
  reciprocals on DVE (InstReciprocal), division = gpsimd partition_broadcast +
  DVE multiply, pipelined into the following pair.
- Tail: pair-3's t=0 division overlaps its t=1 attn pass; proj runs k-inner
  per output tile so tiles stream out add+DMA one by one (both DMA queues) —
  output bandwidth and the division chain finish together.

Numerics: all matmuls bf16 with fp32 PSUM accumulation; everything else fp32.

Self-contained: hardcodes shapes from the problem spec (x (8,512,32,32) f32 etc).
"""
import numpy as np
import ml_dtypes

B, CH, HH, WW = 8, 512, 32, 32
L = HH * WW                  # 1024
HEADS = 8
GROUPS = 32
EPS = 1e-5
DH = CH // HEADS             # 64
KC = CH // 128               # 4 c-chunks
SC = L // 128                # 8 s/l-chunks
TC = L // 512                # 2 t-chunks
GN_N = (CH // GROUPS) * L    # elements per group = 16384

_cache = {}


def _build(has_qkv_bias, has_proj_bias):
    import concourse.bass as bass
    import concourse.tile as tile
    from concourse import bacc, mybir
    import bass_rust as _bass_rust
    from concourse.hw_specs import get_activation_tables

    F32 = mybir.dt.float32
    BF16 = mybir.dt.bfloat16
    F8 = mybir.dt.float8e4
    DR = mybir.MatmulPerfMode.DoubleRow
    AF = mybir.ActivationFunctionType
    OP = mybir.AluOpType
    AX = mybir.AxisListType

    class _Bacc(bacc.Bacc):
        # Pin Exp/Ln to the combined `natural_log_exp_and_others` table set so
        # alternating Ln/Exp activations don't thrash ACT_TABLE_LOADs (~2.7us
        # each).
        def insert_act_table_loads(self):
            has_activation = any(
                isinstance(i, mybir.InstActivation)
                for b in self.main_func.blocks
                for i in b.instructions
            )
            if not has_activation:
                return
            combo = {AF.Exp, AF.Ln}
            tables = []
            for name, fns in get_activation_tables(self.m.arch).items():
                if name != "natural_log_exp_and_others":
                    fns = {f for f in fns if f not in combo}
                tables.append((name, fns))
            _bass_rust.insert_act_table_loads(self, tables)

    nc = _Bacc("TRN2", target_bir_lowering=False, debug=False, num_devices=8)

    x_d = nc.dram_tensor("x", [CH, L], BF16, kind="ExternalInput").ap()
    qw_d = nc.dram_tensor("qkv_wt", [CH, 3 * CH], BF16, kind="ExternalInput").ap()
    pw_d = nc.dram_tensor("proj_wt", [CH, CH], F8, kind="ExternalInput").ap()
    gmask_d = nc.dram_tensor("gmask", [128, 8], F32, kind="ExternalInput").ap()
    gmaskT_d = nc.dram_tensor("gmask_t", [8, 128], F32, kind="ExternalInput").ap()
    if has_qkv_bias:
        qkb_d = nc.dram_tensor("qk_bias", [128, 8], F32, kind="ExternalInput").ap()
        vb_d = nc.dram_tensor("v_bias", [128, KC], F32, kind="ExternalInput").ap()
    if has_proj_bias:
        pb_d = nc.dram_tensor("p_bias", [128, KC], F32, kind="ExternalInput").ap()
    out_d = nc.dram_tensor("out", [CH, L], F32, kind="ExternalOutput").ap()

    with tile.TileContext(nc) as tc:
        import contextlib
        ctx = contextlib.ExitStack()
        pers = ctx.enter_context(tc.tile_pool(name="pers", bufs=1))
        scr = ctx.enter_context(tc.tile_pool(name="scr", bufs=2))
        ewp = ctx.enter_context(tc.tile_pool(name="ewp", bufs=16))
        dvp = ctx.enter_context(tc.tile_pool(name="dvp", bufs=2))
        asg = ctx.enter_context(tc.tile_pool(name="asg", bufs=8))
        outp = ctx.enter_context(tc.tile_pool(name="outp", bufs=3))

        # ---- input DMAs fanned over 4 engine queues (x chunks first so the
        # GN stats pipeline starts asap; qw q/k next; v and pw trail on
        # sync). Each engine's dma_start costs ~0.6us of that engine's time,
        # but ACT/DVE/Pool are idle until the first x chunk lands anyway. ----
        xs = pers.tile([128, KC * L], BF16, tag="xs")
        qw = pers.tile([128, KC * 3 * CH], BF16, tag="qw")
        pw = pers.tile([128, KC * CH], F8, tag="pw")
        gmask = pers.tile([128, 8], F32, tag="gmask")
        gmaskT = pers.tile([8, 128], F32, tag="gmask_t")
        if has_qkv_bias:
            qkb = pers.tile([128, 8], F32, tag="qkb")
            vb = pers.tile([128, KC], F32, tag="vb")
        if has_proj_bias:
            pb = pers.tile([128, KC], F32, tag="pb")
        qeng = [nc.sync, nc.scalar, nc.gpsimd, nc.sync]
        for k in range(KC):
            qeng[k].dma_start(xs[:, k * L:(k + 1) * L],
                              x_d[128 * k:128 * (k + 1), :])
        for k in range(KC):
            qeng[k].dma_start(qw[:, k * 3 * CH:k * 3 * CH + 1024],
                              qw_d[128 * k:128 * (k + 1), 0:1024])
        nc.sync.dma_start(gmask[:], gmask_d[:])
        nc.sync.dma_start(gmaskT[:], gmaskT_d[:])
        if has_qkv_bias:
            nc.sync.dma_start(qkb[:], qkb_d[:])
            nc.sync.dma_start(vb[:], vb_d[:])
        if has_proj_bias:
            nc.sync.dma_start(pb[:], pb_d[:])
        for k in range(KC):   # v columns (vT runs inside pair 0)
            nc.sync.dma_start(qw[:, k * 3 * CH + 1024:(k + 1) * 3 * CH],
                              qw_d[128 * k:128 * (k + 1), 1024:1536])
        for k in range(KC):
            nc.sync.dma_start(pw[:, k * CH:(k + 1) * CH],
                              pw_d[128 * k:128 * (k + 1), :])

        # ---- small constants + ACT table-load trigger ----
        epsb = pers.tile([8, 1], F32, tag="epsb")
        nc.gpsimd.memset(epsb[:], EPS)
        tldt = pers.tile([8, 1], F32, tag="tldt")
        nc.scalar.activation(tldt[:], epsb[:], AF.Exp)

        wsrc = pers.tile([128, 640], BF16, tag="wsrc")
        nc.gpsimd.memset(wsrc[:], 0.0)
        head_psum = tc.tile_pool(name="psH", bufs=1, space="PSUM")
        psH = head_psum.__enter__()
        jps = psH.tile([128, 512], F32, tag="junk", bufs=1)

        def junk(n):
            for _ in range(n):
                nc.tensor.matmul(jps[:], wsrc[:, 0:128], wsrc[:, 128:640],
                                 start=True, stop=True)

        junk(14)

        # ---- GroupNorm stats + xhat per chunk, pair-0 q/k k-outer inline ----
        stat = pers.tile([128, 8], F32, tag="stat")
        xhat = pers.tile([128, KC * L], BF16, tag="xhat")
        bc = pers.tile([128, 2 * KC], F32, tag="bc")
        qk = pers.tile([128, 8 * L], BF16, tag="qk")

        psk = {}
        for j in (0, 4):
            for t in range(TC):
                psk[(j, t)] = psH.tile([128, 512], F32, tag="qkps", bufs=4,
                                       name=f"psk{j}_{t}")

        for k in range(KC):
            xk = xs[:, k * L:(k + 1) * L]
            sq = scr.tile([128, L], F32, tag="sq")
            nc.scalar.activation(sq[:], xk, AF.Square,
                                 accum_out=stat[:, 2 * k + 1:2 * k + 2])
            nc.vector.tensor_reduce(stat[:, 2 * k:2 * k + 1], xk,
                                    axis=AX.X, op=OP.add)
            gst_ps = psH.tile([8, 2], F32, tag="gst", bufs=2)
            nc.tensor.matmul(gst_ps[:], gmask[:], stat[:, 2 * k:2 * k + 2],
                             start=True, stop=True)
            s2k = pers.tile([8, 2], F32, tag=f"s2k{k}")
            vk = pers.tile([8, 2], F32, tag=f"vk{k}")
            nc.vector.tensor_scalar_mul(s2k[:], gst_ps[:], 1.0 / GN_N)
            nc.vector.tensor_mul(vk[:, 1:2], s2k[:, 0:1], s2k[:, 0:1])
            nc.vector.tensor_sub(vk[:, 0:1], s2k[:, 1:2], vk[:, 1:2])
            nc.scalar.activation(vk[:, 1:2], vk[:, 0:1], AF.Ln, bias=epsb[:])
            nc.scalar.activation(s2k[:, 1:2], vk[:, 1:2], AF.Exp, scale=-0.5)
            bc_ps = psH.tile([128, 2], F32, tag="bcps", bufs=1)
            nc.tensor.matmul(bc_ps[:], gmaskT[:], s2k[:], start=True, stop=True)
            nc.vector.tensor_copy(bc[:, 2 * k:2 * k + 2], bc_ps[:])
            nmr = pers.tile([128, 1], F32, tag=f"nmr{k}")
            nc.vector.tensor_scalar(
                out=nmr[:], in0=bc[:, 2 * k:2 * k + 1],
                scalar1=bc[:, 2 * k + 1:2 * k + 2], scalar2=-1.0,
                op0=OP.mult, op1=OP.mult)
            nc.vector.tensor_scalar(
                out=xhat[:, k * L:(k + 1) * L], in0=xk,
                scalar1=bc[:, 2 * k + 1:2 * k + 2], scalar2=nmr[:],
                op0=OP.mult, op1=OP.add)
            for j in (0, 4):
                for t in range(TC):
                    nc.tensor.matmul(
                        psk[(j, t)][:],
                        qw[:, k * 3 * CH + 128 * j:k * 3 * CH + 128 * (j + 1)],
                        xhat[:, k * L + 512 * t:k * L + 512 * (t + 1)],
                        start=(k == 0), stop=(k == KC - 1))
            junk(2)

        def qk_store(j, t, ps_ap, eng):
            dst = qk[:, j * L + 512 * t:j * L + 512 * (t + 1)]
            if eng == "s":
                if has_qkv_bias:
                    nc.scalar.activation(dst, ps_ap, AF.Identity,
                                         bias=qkb[:, j:j + 1])
                else:
                    nc.scalar.activation(dst, ps_ap, AF.Identity)
            else:
                if has_qkv_bias:
                    nc.vector.tensor_scalar_add(dst, ps_ap, qkb[:, j:j + 1])
                else:
                    nc.vector.tensor_copy(dst, ps_ap)

        qk_store(0, 0, psk[(0, 0)][:], "v")
        qk_store(4, 0, psk[(4, 0)][:], "s")
        qk_store(0, 1, psk[(0, 1)][:], "v")
        qk_store(4, 1, psk[(4, 1)][:], "s")
        head_psum.__exit__(None, None, None)

        # ---- attention phase pools: scores 4 banks, attn accumulators 2
        # banks (two t-passes), filler accumulators 2 banks.
        # v/ew are fp8e4: exp(score-4) fits e4m3 (|score|<6.3), the -4 shift
        # cancels in the softmax division, and fp8 enables DoubleRow attn
        # matmuls (2 sc-chunks contracted per matmul at 0.5 cyc/row). ----
        a_sb = pers.tile([128, KC * L], F8, tag="a_sb")
        # vt layout: [scp][head][sc-even 80 | sc-odd 80] (65 used + 15 pad
        # per group) — dual-fp8 LDWEIGHTS needs the two contraction groups at
        # an even, 16B-aligned stride
        vt = pers.tile([128, (SC // 2) * HEADS * 160], F8, tag="vt")
        for scp in range(SC // 2):
            v4 = vt[:, scp * 1280:(scp + 1) * 1280].rearrange(
                "p (h s c) -> p h s c", h=8, s=2, c=80)
            nc.gpsimd.memset(v4[:, :, :, 64:65], 1.0)
        nb4 = pers.tile([128, 1], F32, tag="nb4")
        nc.gpsimd.memset(nb4[:], -4.0)

        attn_psum = tc.tile_pool(name="psS", bufs=2, space="PSUM")
        psS = attn_psum.__enter__()
        attn_acc = tc.tile_pool(name="psA", bufs=2, space="PSUM")
        psA = attn_acc.__enter__()
        fill_psum = tc.tile_pool(name="psF", bufs=2, space="PSUM")
        psF = fill_psum.__enter__()

        vt_ps = {}

        def emit_vt_q(lc, k):
            # vT for one l-chunk, one k-chunk matmul per quarter emission
            if k == 0:
                vt_ps[lc] = psF.tile([128, 512], F32, tag="f", bufs=2,
                                     name=f"vtps{lc}")
            ps = vt_ps[lc]
            nc.tensor.matmul(
                ps[:], xhat[:, k * L + 128 * lc:k * L + 128 * (lc + 1)],
                qw[:, k * 3 * CH + 2 * CH:k * 3 * CH + 3 * CH],
                start=(k == 0), stop=(k == KC - 1))
            if k == KC - 1:
                # cast to fp8 on the way into vt ([scp][h][so 80|80] layout)
                scp, so = divmod(lc, 2)
                v4_ = vt[:, scp * 1280:(scp + 1) * 1280].rearrange(
                    "p (hh s c) -> p hh s c", hh=8, s=2, c=80)
                src = ps[:].rearrange("p (hh c) -> p hh c", c=64)
                nc.vector.tensor_copy(v4_[:, :, so, 0:64], src)

        fq_ps = {}

        def emit_qk_q(j, k):
            # one k-chunk (both t halves) of a later-pair q/k o-chunk per
            # quarter emission; stores ride the last quarter
            if k == 0:
                fq_ps[j] = [psF.tile([128, 512], F32, tag="f", bufs=2,
                                     name=f"fq{j}_{t}") for t in range(TC)]
            for t in range(TC):
                nc.tensor.matmul(
                    fq_ps[j][t][:],
                    qw[:, k * 3 * CH + 128 * j:k * 3 * CH + 128 * (j + 1)],
                    xhat[:, k * L + 512 * t:k * L + 512 * (t + 1)],
                    start=(k == 0), stop=(k == KC - 1))
            if k == KC - 1:
                for t in range(TC):
                    qk_store(j, t, fq_ps[j][t][:], "v")

        def V(lc, k):
            return lambda: emit_vt_q(lc, k)

        def Q(j, k):
            return lambda: emit_qk_q(j, k)

        fillers = {
            (0, 1): [V(0, 0), V(0, 1), V(0, 2), V(0, 3), V(1, 0), V(1, 1)],
            (0, 2): [V(1, 2), V(1, 3), V(2, 0), V(2, 1), V(2, 2), V(2, 3)],
            (0, 3): [V(3, 0), V(3, 1), V(3, 2), V(3, 3)],
            (0, 4): [V(4, 0), V(4, 1), V(4, 2), V(4, 3), Q(1, 0), Q(1, 1),
                     Q(5, 0)],
            (0, 5): [V(5, 0), V(5, 1), V(5, 2), V(5, 3), Q(1, 2), Q(1, 3),
                     Q(5, 1)],
            (0, 6): [V(6, 0), V(6, 1), V(6, 2), V(6, 3), Q(5, 2), Q(5, 3)],
            (0, 7): [V(7, 0), V(7, 1), V(7, 2), V(7, 3)],
            (1, 3): [Q(2, 0), Q(2, 1)],
            (1, 4): [Q(2, 2), Q(2, 3)],
            (1, 5): [Q(6, 0), Q(6, 1)],
            (1, 6): [Q(6, 2), Q(6, 3)],
            (2, 3): [Q(3, 0), Q(3, 1)],
            (2, 4): [Q(3, 2), Q(3, 3)],
            (2, 5): [Q(7, 0), Q(7, 1)],
            (2, 6): [Q(7, 2), Q(7, 3)],
        }

        def q_ap(m, e, t):
            return qk[64 * e:64 * (e + 1), m * L + 512 * t:m * L + 512 * (t + 1)]

        def k_ap(m, e, sc):
            return qk[64 * e:64 * (e + 1),
                      (4 + m) * L + 128 * sc:(4 + m) * L + 128 * (sc + 1)]

        # ew4[(m, scp, e)]: [128, 2048] fp8 tile for the sc pair {2scp, 2scp+1}
        # laid out [scE-t0 | scO-t0 | scE-t1 | scO-t1] so a DoubleRow attn
        # matmul reads a contiguous [128, 2, 512] rhs per t
        ew_tiles = {}

        def emit_scores(m, sc):
            scp, so = divmod(sc, 2)
            ps_w = [None, None]
            for e in range(2):
                pw_t = psS.tile([128, 1024], F32, tag="ps")
                ps_w[e] = pw_t
            for e in range(2):
                for t in range(TC):
                    nc.tensor.matmul(ps_w[e][:, 512 * t:512 * (t + 1)],
                                     k_ap(m, e, sc), q_ap(m, e, t),
                                     start=True, stop=True)
            for e in range(2):
                if so == 0:
                    ew_tiles[(m, scp, e)] = ewp.tile(
                        [128, 2 * L], F8, tag="ew", name=f"ew{m}_{scp}_{e}")
                ew4 = ew_tiles[(m, scp, e)]
                dst = ew4.rearrange("p (t s c) -> p t s c", t=2, s=2,
                                    c=512)[:, :, so, :]
                nc.scalar.activation(dst, ps_w[e][:], AF.Exp, bias=nb4[:])

        pa = {}
        psA3 = [None]  # pair-3 t1 accumulator pool, opened once psF retires

        def attn_dr(m, tp, scp, e):
            # DoubleRow fp8 matmul: contracts sc-chunks 2scp and 2scp+1 at
            # once (vt lhsT [128, 2, 65], ew rhs [128, 2, 512])
            if (m, tp) not in pa:
                if (m, tp) == (3, 1):
                    pa[(m, tp)] = [psA3[0].tile([65, 512], F32, tag="pa3",
                                                name=f"pa3_1_{ee}")
                                   for ee in range(2)]
                else:
                    pa[(m, tp)] = [psA.tile([65, 512], F32, tag="pa",
                                            name=f"pa{m}_{tp}_{ee}")
                                   for ee in range(2)]
            h0 = scp * 1280 + (2 * m + e) * 160
            lhsT = vt[:, h0:h0 + 160].rearrange(
                "p (s c) -> p s c", c=80)[:, :, 0:65]
            rhs = ew_tiles[(m, scp, e)][:, tp * 1024:(tp + 1) * 1024].rearrange(
                "p (s c) -> p s c", s=2)
            nc.tensor.matmul(
                pa[(m, tp)][e][:], lhsT, rhs,
                start=(scp == 0), stop=(scp == SC // 2 - 1),
                perf_mode=DR)

        def div_recip2(stgs2, engs=None):
            # copy ones-row to partition 0, then approx reciprocal (both ops
            # require partition-0-aligned operands); no DMA round trip
            rdens = []
            for i, (sg, e, t, mm_) in enumerate(stgs2):
                den = dvp.tile([1, 512], F32, tag="den", bufs=4,
                               name=f"den{mm_}_{t}_{e}")
                nc.vector.tensor_copy(den[:], sg[64:65, :])
                r = dvp.tile([1, 512], F32, tag="rden", bufs=4,
                             name=f"r{mm_}_{t}_{e}")
                nc.vector.reciprocal_approx_fast(r[:], den[:])
                rdens.append(r)
            return rdens

        def div_mul(rdens, i, sg, e, t, mm_, mul_eng="v"):
            bsb = dvp.tile([64, 512], F32, tag="bsb", bufs=4)
            nc.gpsimd.partition_broadcast(bsb[:], rdens[i][0:1, :])
            dst = a_sb[64 * e:64 * (e + 1),
                       mm_ * L + 512 * t:mm_ * L + 512 * (t + 1)]
            eng = nc.vector if mul_eng == "v" else nc.gpsimd
            eng.tensor_mul(dst, sg[0:64, :], bsb[:])
            if has_qkv_bias:
                nc.vector.tensor_scalar_add(
                    dst, dst, vb[64 * e:64 * (e + 1), mm_:mm_ + 1])

        def stage(m, tp, engs=("v", "v")):
            out = []
            for e in range(2):
                sg = asg.tile([65, 512], F32, tag="astg", name=f"sg{m}_{tp}_{e}")
                if engs[e] == "s":
                    nc.scalar.activation(sg[:], pa[(m, tp)][e][:], AF.Identity)
                else:
                    nc.vector.tensor_copy(sg[:], pa[(m, tp)][e][:])
                out.append((sg, e, tp, m))
            return out

        def tail_steps(m):
            # previous pair's tail, spread one step per (m+1, sc) slot so it
            # never lumps in the PE FIFO ahead of the next pair's scores
            for e in range(2):
                attn_dr(m, 0, 3, e)
            stgs_t0 = stage(m, 0)
            yield
            for scp in (0, 1):
                for e in range(2):
                    attn_dr(m, 1, scp, e)
            yield
            for scp in (2, 3):
                for e in range(2):
                    attn_dr(m, 1, scp, e)
            stgs_t1 = stage(m, 1)
            yield
            rden0 = div_recip2(stgs_t0)
            yield
            for i, s in enumerate(stgs_t0):
                div_mul(rden0, i, *s)
            yield
            rden1 = div_recip2(stgs_t1)
            yield
            for i, s in enumerate(stgs_t1):
                div_mul(rden1, i, *s)
            yield

        attn_acc3 = None
        pending_tail = None
        for g in range(32):
            m, sc = divmod(g, 8)
            if g == 24:
                # fillers are done (last at (2,6)); retire psF and reuse its
                # 2 banks for pair-3's t1 accumulators so the t1 attn pass
                # runs lag-2 in-loop instead of entirely after the last exp
                fill_psum.__exit__(None, None, None)
                attn_acc3 = tc.tile_pool(name="psA3", bufs=2, space="PSUM")
                psA3[0] = attn_acc3.__enter__()
            emit_scores(m, sc)
            if sc == 0:
                if pending_tail is not None:
                    for _ in pending_tail:
                        pass
                pending_tail = tail_steps(m - 1) if m >= 1 else None
            if pending_tail is not None:
                next(pending_tail, None)
            for f in fillers.get((m, sc), ()):
                f()
            if sc in (3, 5, 7):
                scp = (sc - 3) // 2
                for e in range(2):
                    attn_dr(m, 0, scp, e)
                if m == 3:
                    for e in range(2):
                        attn_dr(m, 1, scp, e)

        # ---- pair-3 tail: both t-passes already ran lag in-loop; only
        # sc-pair 3 remains. No staging — numerators and the ones-row are
        # read straight from the attn PSUM accumulators; the den/recip/
        # bcast/mul chain is software-pipelined across the (tp, e) items. ----
        if pending_tail is not None:
            for _ in pending_tail:
                pass
        for e in range(2):
            attn_dr(3, 0, 3, e)
        for e in range(2):
            attn_dr(3, 1, 3, e)

        items = [(0, 0), (0, 1), (1, 0), (1, 1)]   # (tp, e), t0 first (proj t0)
        rs, bs = {}, {}

        def t3_den(tp, e):
            den = dvp.tile([1, 512], F32, tag="den", bufs=4,
                           name=f"d3_{tp}_{e}")
            nc.vector.tensor_copy(den[:], pa[(3, tp)][e][64:65, :])
            r = dvp.tile([1, 512], F32, tag="rden", bufs=4, name=f"r3_{tp}_{e}")
            nc.vector.reciprocal_approx_fast(r[:], den[:])
            rs[(tp, e)] = r

        def t3_bcast(tp, e):
            bsb = dvp.tile([64, 512], F32, tag="bsb", bufs=4,
                           name=f"b3_{tp}_{e}")
            nc.gpsimd.partition_broadcast(bsb[:], rs[(tp, e)][0:1, :])
            bs[(tp, e)] = bsb

        def t3_mul(tp, e):
            dst = a_sb[64 * e:64 * (e + 1),
                       3 * L + 512 * tp:3 * L + 512 * (tp + 1)]
            nc.vector.tensor_mul(dst, pa[(3, tp)][e][0:64, :], bs[(tp, e)][:])
            if has_qkv_bias:
                nc.vector.tensor_scalar_add(
                    dst, dst, vb[64 * e:64 * (e + 1), 3:4])

        # all dens+recips first (bcasts overlap the later DVE ops), then muls
        t3_den(0, 0)
        t3_bcast(0, 0)
        t3_den(0, 1)
        t3_bcast(0, 1)
        t3_den(1, 0)
        t3_bcast(1, 0)
        t3_den(1, 1)
        t3_bcast(1, 1)
        t3_mul(0, 0)
        t3_mul(0, 1)
        t3_mul(1, 0)
        t3_mul(1, 1)
        if attn_acc3 is not None:
            attn_acc3.__exit__(None, None, None)
        attn_acc.__exit__(None, None, None)
        attn_psum.__exit__(None, None, None)

        # ---- proj + residual. The k=0..2 partial accumulations only need
        # pairs 0-2 of a_sb (divided in-stream long ago), so they run DURING
        # the pair-3 division window — this keeps the PE HAM clock warm and
        # leaves only the k=3 finisher per tile gated on pair-3's muls. All 8
        # accumulators live at once (PSUM is free after the attn pools). ----
        # bufs=4: exactly the 4 banks the score pool vacated at the last exp
        # — overlapping the attn-accumulator banks would make the first proj
        # write wait for pair-3's division reads (tile-level WAR)
        with tc.tile_pool(name="psP", bufs=4, space="PSUM") as psP:
            ps_tiles = {}

            def proj_k012(t):
                for i in range(KC):
                    ps = psP.tile([128, 512], F32, tag="ps", name=f"psp{t}_{i}")
                    ps_tiles[(t, i)] = ps
                    pw_k = pw.rearrange("p (kk c) -> p kk c", c=CH)
                    ab_k = a_sb.rearrange("p (kk c) -> p kk c", c=L)
                    lhsT = pw_k[:, 0:2, 128 * i:128 * (i + 1)]
                    rhs = ab_k[:, 0:2, 512 * t:512 * (t + 1)]
                    nc.tensor.matmul(ps[:], lhsT, rhs,
                                     start=True, stop=False, perf_mode=DR)

            idx = 0

            def proj_fin(t):
                nonlocal idx
                kp = 2
                pw_k = pw.rearrange("p (kk c) -> p kk c", c=CH)
                ab_k = a_sb.rearrange("p (kk c) -> p kk c", c=L)
                for i in range(KC):
                    ps = ps_tiles[(t, i)]
                    lhsT = pw_k[:, kp:kp + 2, 128 * i:128 * (i + 1)]
                    rhs = ab_k[:, kp:kp + 2, 512 * t:512 * (t + 1)]
                    nc.tensor.matmul(ps[:], lhsT, rhs,
                                     start=False, stop=True, perf_mode=DR)
                    ot = outp.tile([128, 512], F32, tag="ot")
                    nc.vector.tensor_add(ot[:],
                                         xs[:, i * L + 512 * t:i * L + 512 * (t + 1)],
                                         ps[:])
                    if has_proj_bias:
                        nc.vector.tensor_scalar_add(ot[:], ot[:], pb[:, i:i + 1])
                    eng = (nc.sync, nc.scalar)[idx % 2]
                    eng.dma_start(
                        out_d[128 * i:128 * (i + 1), 512 * t:512 * (t + 1)], ot[:])
                    idx += 1

            proj_k012(0)
            proj_k012(1)
            proj_fin(0)
            proj_fin(1)
        ctx.close()

    nc.compile()
    return nc


def _prep_inputs(x, norm_w, norm_b, qkv_w, qkv_b, proj_w, proj_b):
    scale = DH ** -0.25
    w_eff = (qkv_w.astype(np.float64) * norm_w.astype(np.float64)[None, :])
    b_eff = qkv_b.astype(np.float64) + w_eff @ norm_b.astype(np.float64)
    perm = np.concatenate([
        np.concatenate([np.arange(h * 3 * DH + t * DH, h * 3 * DH + (t + 1) * DH)
                        for h in range(HEADS)])
        for t in range(3)])
    w_eff = w_eff[perm]
    b_eff = b_eff[perm]
    w_eff[:2 * CH] *= scale
    b_eff[:2 * CH] *= scale
    qkv_wt = np.ascontiguousarray(w_eff.T).astype(np.float32).astype(
        ml_dtypes.bfloat16)
    proj_wt = np.ascontiguousarray(proj_w.T).astype(np.float32).astype(
        ml_dtypes.float8_e4m3)

    p = np.arange(128)
    gmask = (p[:, None] // 16 == np.arange(8)[None, :]).astype(np.float32)
    gmask_t = np.ascontiguousarray(gmask.T)

    has_qkv_bias = bool(np.any(b_eff != 0.0))
    has_proj_bias = bool(np.any(proj_b != 0.0))
    common = {"qkv_wt": qkv_wt, "proj_wt": proj_wt, "gmask": gmask,
              "gmask_t": gmask_t}
    if has_qkv_bias:
        qk_part = b_eff[:2 * CH].astype(np.float32).reshape(8, 128).T
        v_part = b_eff[2 * CH:].astype(np.float32).reshape(KC, 128).T
        common["qk_bias"] = np.ascontiguousarray(qk_part)
        common["v_bias"] = np.ascontiguousarray(v_part)
    if has_proj_bias:
        common["p_bias"] = np.ascontiguousarray(
            proj_b.astype(np.float32).reshape(KC, 128).T)
    xf = np.ascontiguousarray(x.reshape(B, CH, L)).astype(np.float32)
    xf16 = xf.astype(ml_dtypes.bfloat16)
    in_maps = [dict(common, x=np.ascontiguousarray(xf16[i])) for i in range(B)]
    return in_maps, has_qkv_bias, has_proj_bias


def _get_nc(flags):
    if flags not in _cache:
        _cache[flags] = _build(*flags)
    return _cache[flags]


def _run(inputs, trace=False, tmpdir=None):
    import time
    from concourse.bass_utils import run_bass_kernel_spmd
    in_maps, hqb, hpb = _prep_inputs(**inputs)
    nc = _get_nc((hqb, hpb))
    kw = {}
    if trace:
        kw = dict(trace=True, tmpdir=tmpdir)
    last_err = None
    for attempt in range(3):
        try:
            res = run_bass_kernel_spmd(nc, in_maps, list(range(B)), **kw)
            break
        except Exception as e:  # noqa: BLE001
            last_err = e
            time.sleep(5)
    else:
        raise last_err
    out = np.stack([res.results[i]["out"] for i in range(B)])
    return out.reshape(B, CH, HH, WW).astype(np.float32), res


def kernel(x, norm_w, norm_b, qkv_w, qkv_b, proj_w, proj_b):
    out, _ = _run(dict(x=x, norm_w=norm_w, norm_b=norm_b, qkv_w=qkv_w,
                       qkv_b=qkv_b, proj_w=proj_w, proj_b=proj_b))
    return out

